# revision 7
# baseline (speedup 1.0000x reference)
"""Multi-head attention (B=2, T=2048, C=1024, H=16) on 8 TRN2 NeuronCores.

Sharding: core c = (b, g) with b = c // 4 (data parallel over batch),
g = c % 4 (tensor parallel over head groups of 4 heads = 256 cols).
Wq/Wk/Wv column-sharded, Wp row-sharded (Megatron); host sums the 4
partial y's per batch and adds the bias.

Per-core pipeline (single TileContext, phases overlap via tile deps):
  A: Q/K projections in fp8 DoubleRow (x and 8*W quantized to e4m3;
     the head-dim is split as partition=h*32+(d%32), k-tile=d//32 via a
     host-side column permutation of the weights, so evacuation is a
     single strided scale-copy).  V projection in bf16.  Q(qh0) first,
     then K/V streamed per 512-token chunk with the matching (qh0, hp0)
     attention chunks interleaved so the ACT engine starts early.
  B: per (qh, head-pair, kc): S^T chunks via fp8 DoubleRow (4 matmuls of
     256 columns) -> exp on ACT with the dequant scale folded in (ACT
     does nothing else all kernel) -> mask multiply on DVE -> PV with
     V augmented by a ones column so row 64 of O^T is the denominator.
     Normalization: reciprocal row -> K=1 broadcast matmul -> one fused
     multiply that also evacuates O^T to bf16 SBUF.  Odd heads travel
     through a small SBUF->SBUF DMA to land at partitions 64..127 so
     phase C can contract head pairs with K=128.
  C (tail): y = O^T_pair.T @ Wp_pair, K=128, y stored bf16.
"""
import numpy as np
import ml_dtypes

import bass_rust
import concourse.bass as bass
import concourse.mybir as mybir
import concourse.tile as tile
from concourse.bass_utils import run_bass_kernel_spmd
from concourse.vector_clock import ScopedClock

# ---------------------------------------------------------------------------
# Workaround: walrus rejects >~4 sync waits on one instruction; the Tile exit
# drain aggregates one wait per DMA queue/engine.  Spread them over a chain of
# single-wait NOPs on the sync engine before draining.
# ---------------------------------------------------------------------------


def _patched_drain_and_barrier(self, tick_clock, wait_clock):
    nc = self.nc
    probe = nc.sync.nop(nofuse=True)
    wait_clock.add_sem_waits(probe.ins, ScopedClock({None: tick_clock.global_clock}))
    waits = list(probe.ins.sync_info.on_wait) if probe.ins.sync_info else []
    probe.ins.sync_info = bass_rust.SyncInfo(
        on_wait=waits[:1], on_update=[]
    )
    for w in waits[1:]:
        n = nc.sync.nop(nofuse=True)
        n.ins.sync_info = bass_rust.SyncInfo(on_wait=[w], on_update=[])

    nc.sync.drain()
    nc.all_engine_barrier()
    assert self.sems is not None
    popped = nc._tile_sem_poison_stack.pop()
    assert popped is self._sem_poison
    nc.clear_and_free_semaphores(list(self.sems.allocated().values()))
    nc.all_engine_barrier()


tile.TileContext._drain_and_barrier = _patched_drain_and_barrier

_MAX_WAITS = 1


def _split_excess_waits(nc, limit=_MAX_WAITS):
    """Walrus codegen allows only ONE sync wait on compute instructions.
    For any instruction carrying more, peel the excess onto same-engine
    single-wait NOPs inserted immediately before it in the basic block."""
    n_new = 0
    for f in nc.m.functions:
        for bb in f.blocks:
            insts = bb.instructions
            out = []
            for inst in insts:
                si = inst.sync_info
                waits = list(si.on_wait) if si and si.on_wait else []
                if len(waits) > limit:
                    extra, keep = waits[:-limit], waits[-limit:]
                    inst.sync_info = bass_rust.SyncInfo(
                        on_wait=keep, on_update=list(si.on_update)
                    )
                    for j in range(0, len(extra), limit):
                        nop = mybir.InstNoOp(
                            name=f"waitsplit-{n_new}",
                            engine=inst.engine,
                            ins=[],
                            outs=[],
                            sync_info=bass_rust.SyncInfo(
                                on_wait=extra[j:j + limit], on_update=[]
                            ),
                        )
                        n_new += 1
                        out.append(nop)
                out.append(inst)
            if n_new:
                bb.instructions = out
    return n_new

# ---------------------------------------------------------------------------

B, T, C, H = 2, 2048, 1024, 16
GROUPS = 4                 # head groups (tensor parallel width per batch)
HG = H // GROUPS           # 4 heads per group
DH = C // H                # 64
COLS = HG * DH             # 256 local columns
KC = T // 128              # 16 k-chunks of 128
CC = C // 128              # 8 contraction chunks for the projections
QC = T // 512              # 4 token chunks of 512 for projections

F32 = mybir.dt.float32
F32R = mybir.dt.float32r
BF16 = mybir.dt.bfloat16
FP8 = mybir.dt.float8e4
DR = mybir.MatmulPerfMode.DoubleRow

# qt8/kt8 store 2*q_true; S_raw = 4*q.k, so exp scale = C^-0.5 / 4
EXP_SCALE = float(C) ** -0.5 / 4.0
EVAC_SCALE = 0.25          # psum (x8 @ 8W) -> store 2*q_true


def _mm(nc, out, lhsT, rhs, start, stop, perf_mode=None, tile_position=None,
        skip_group_check=False):
    nc.tensor.matmul(out, lhsT, rhs, start=start, stop=stop,
                     perf_mode=perf_mode, tile_position=tile_position,
                     skip_group_check=skip_group_check)


def build_program(split_waits=True):
    nc = bass.Bass("TRN2", target_bir_lowering=False, debug=False, num_devices=8)

    xq8 = nc.declare_dram_parameter("xq8", [128, CC // 2, 2, T], FP8, isOutput=False)
    xk8 = nc.declare_dram_parameter("xk8", [128, CC // 2, 2, T], FP8, isOutput=False)
    xvT = nc.declare_dram_parameter("xvT", [C, T], BF16, isOutput=False)
    maskT = nc.declare_dram_parameter("maskT", [T, T], BF16, isOutput=False)
    wq8 = nc.declare_dram_parameter("wq8", [128, CC // 2, 2, 2, 128], FP8,
                                    isOutput=False)
    wk8 = nc.declare_dram_parameter("wk8", [128, CC // 2, 2, 2, 128], FP8,
                                    isOutput=False)
    wv = nc.declare_dram_parameter("wv", [C, COLS], BF16, isOutput=False)
    wp = nc.declare_dram_parameter("wp", [128, 2, C], BF16, isOutput=False)
    y = nc.declare_dram_parameter("y", [T, C], BF16, isOutput=True)

    with tile.TileContext(nc) as tc:
        import contextlib
        with contextlib.ExitStack() as ctx:
            persist = ctx.enter_context(tc.tile_pool(name="persist", bufs=1))

            # persistent SBUF tensors
            mask_sb = persist.tile([128, KC, T], BF16)          # 64 KB/part
            qt8_sb = persist.tile([128, 2, T], FP8)             # 4 KB/part
            kt8_sb = persist.tile([128, 2, T], FP8)             # 4 KB/part
            vaug_sb = persist.tile([128, KC, HG, DH + 1], BF16)  # 8.1 KB
            otp_sb = persist.tile([128, 2, T], BF16)            # 8 KB
            wq8_sb = persist.tile([128, CC // 2, 2, 2, 128], FP8)  # 2 KB
            wk8_sb = persist.tile([128, CC // 2, 2, 2, 128], FP8)
            wv_sb = persist.tile([128, CC, COLS], BF16)         # 4 KB
            wp_sb = persist.tile([128, 2, C], BF16)             # 4 KB
            ones_sb = persist.tile([1, DH], F32R)

            nc.vector.memset(vaug_sb[:, :, :, DH:DH + 1], 1.0)
            nc.vector.memset(ones_sb.bitcast(F32), 1.0)
            nc.gpsimd.dma_start(wq8_sb, wq8[:, :, :, :, :])
            nc.gpsimd.dma_start(wk8_sb, wk8[:, :, :, :, :])

            px = ctx.enter_context(tc.tile_pool(name="xchunks", bufs=4))
            ppt = ctx.enter_context(tc.tile_pool(name="pt", bufs=13))
            prc = ctx.enter_context(tc.tile_pool(name="recip", bufs=2))
            ptmp = ctx.enter_context(tc.tile_pool(name="tmp", bufs=2))
            py = ctx.enter_context(tc.tile_pool(name="ysb", bufs=5))
            pps = ctx.enter_context(
                tc.tile_pool(name="psum_s", bufs=2, space="PSUM"))
            ppv = ctx.enter_context(
                tc.tile_pool(name="psum_ot", bufs=2, space="PSUM"))

            # ---------------- Phase A helpers ----------------
            def proj_qk8(x8_dram, w8_sb, out_sb, qc):
                """fp8 DoubleRow projection of 512 tokens into out_sb."""
                qs = slice(qc * 512, (qc + 1) * 512)
                xt = px.tile([128, CC // 2, 2, 512], FP8, tag="x8", name="x8")
                nc.gpsimd.dma_start(xt, x8_dram[:, :, :, qs])
                ps = pps.tile([128, 2, 512], F32, tag="s", name="qk_ps")
                for j in range(2):
                    for ccp in range(CC // 2):
                        for th in range(2):
                            # PSUM zeroing is bank-granular: only the first
                            # write to the bank may carry start=True
                            st = ccp == 0 and th == 0
                            sp = ccp == CC // 2 - 1 and th == 1
                            _mm(nc, ps[:, j, th * 256:(th + 1) * 256],
                                w8_sb[:, ccp, :, j, :],
                                xt[:, ccp, :, th * 256:(th + 1) * 256],
                                st, sp, perf_mode=DR,
                                skip_group_check=True)
                nc.vector.tensor_scalar_mul(out_sb[:, :, qs], ps, EVAC_SCALE)

            def proj_v(qc):
                qs = slice(qc * 512, (qc + 1) * 512)
                xt = px.tile([128, CC, 512], BF16, tag="xv", name="xv")
                half = (CC // 2) * 128
                nc.gpsimd.dma_start(
                    xt[:, 0:CC // 2],
                    xvT[0:half, qs].rearrange("(cc p) q -> p cc q", p=128))
                nc.gpsimd.dma_start(
                    xt[:, CC // 2:CC],
                    xvT[half:C, qs].rearrange("(cc p) q -> p cc q", p=128))
                for tp in range(2):
                    ps = pps.tile([128, 2, 512], F32, tag="s", name="v_ps")
                    for cc in range(CC):
                        st, sp = cc == 0, cc == CC - 1
                        for ti in range(2):
                            tt = tp * 2 + ti
                            _mm(nc, ps[:, ti, 0:COLS],
                                xt[:, cc, tt * 128:(tt + 1) * 128],
                                wv_sb[:, cc], st, sp)
                    for ti in range(2):
                        tt = tp * 2 + ti
                        nc.vector.tensor_copy(
                            vaug_sb[:, qc * 4 + tt, :, 0:DH],
                            ps[:, ti, 0:COLS].rearrange("p (h d) -> p h d", h=HG))

            def load_mask(kc2, qh):
                qsl = slice(qh * 1024, (qh + 1) * 1024)
                nc.gpsimd.dma_start(
                    mask_sb[:, kc2:kc2 + 2, qsl],
                    maskT[kc2 * 128:(kc2 + 2) * 128, qsl].rearrange(
                        "(c p) q -> p c q", p=128))

            # ---------------- Phase B building blocks ----------------
            def b_smask(qh, hp, kc, h2):
                """S (fp8 DR) -> exp -> mask; returns the masked-P tile."""
                qsl = slice(qh * 1024, (qh + 1) * 1024)
                ks = slice(kc * 128, (kc + 1) * 128)
                h = 2 * hp + h2
                hs = slice(h * 32, (h + 1) * 32)
                s_t = pps.tile([128, 1024], F32, tag="s", name="s_t")
                for blk in range(4):
                    qq = slice(qh * 1024 + blk * 256,
                               qh * 1024 + (blk + 1) * 256)
                    _mm(nc, s_t[:, blk * 256:(blk + 1) * 256],
                        kt8_sb[hs, :, ks], qt8_sb[hs, :, qq],
                        blk % 2 == 0, blk % 2 == 1, perf_mode=DR,
                        tile_position=(h * 32, 0), skip_group_check=True)
                pt_t = ppt.tile([128, 1024], BF16, tag="pt", name="pt_t")
                nc.scalar.activation(
                    pt_t, s_t, mybir.ActivationFunctionType.Exp,
                    scale=EXP_SCALE)
                nc.vector.tensor_mul(pt_t, pt_t, mask_sb[:, kc, qsl])
                return pt_t

            def b_pv(hp, kc, h2, ot, pt_t):
                h = 2 * hp + h2
                for jj in range(2):
                    _mm(nc, ot[h2][:, jj * 512:(jj + 1) * 512],
                        vaug_sb[:, kc, h],
                        pt_t[:, jj * 512:(jj + 1) * 512],
                        kc == 0, kc == KC - 1)

            # Software pipeline: each PV is emitted PV_LAG h2-steps after its
            # S/exp/mask so PV never parks in PE's 4-deep in-order wait
            # queue waiting for the DVE mask multiply (which would convoy
            # the next chunk's S matmuls and starve the ACT engine).
            PV_LAG = 4
            pend_pv = []

            def b_step(qh, hp, kc, h2, ot, lag=PV_LAG):
                pt_t = b_smask(qh, hp, kc, h2)
                pend_pv.append((hp, kc, h2, ot, pt_t))
                while len(pend_pv) > lag:
                    b_pv(*pend_pv.pop(0))

            def b_drain(limit, exclude_ot=None):
                while len(pend_pv) > limit and (
                        exclude_ot is None
                        or pend_pv[0][3] is not exclude_ot):
                    b_pv(*pend_pv.pop(0))

            def b_flush():
                b_drain(0)

            def b_chunk(qh, hp, kcs, ot):
                """S -> exp -> mask -> (lagged) PV for k-chunks kcs."""
                for kc in kcs:
                    for h2 in range(2):
                        b_step(qh, hp, kc, h2, ot)

            def b_recips(ot):
                """reciprocal of the denominator rows (cheap, unblocks the
                broadcast matmuls emitted later)."""
                rcs = []
                for h2 in range(2):
                    rc = prc.tile([1, 1024], F32R, tag="rc", name="rc")
                    with nc.allow_low_precision(reason="softmax denom recip"):
                        nc.vector.reciprocal(rc, ot[h2][64:65, :])
                    rcs.append(rc)
                return rcs

            def b_norm(qh, hp, ot, rcs):
                """broadcast the reciprocal row with a K=1 matmul, copy it
                to SBUF (walrus allows one PSUM input per vector op), then
                one fused normalize/evacuate multiply; odd head shifts
                partitions via a small SBUF->SBUF DMA."""
                qsl = slice(qh * 1024, (qh + 1) * 1024)
                for h2 in range(2):
                    bc = pps.tile([64, 1024], F32, tag="s", name="bc")
                    for jj in range(2):
                        _mm(nc, bc[:, jj * 512:(jj + 1) * 512], ones_sb,
                            rcs[h2][:, jj * 512:(jj + 1) * 512], True, True)
                    bc_sb = ptmp.tile([64, 1024], F32R, tag="bc", name="bc_sb")
                    nc.vector.tensor_copy(bc_sb, bc)
                    if h2 == 0:
                        nc.vector.tensor_mul(otp_sb[0:64, hp, qsl],
                                             ot[h2][0:64, :], bc_sb)
                    else:
                        tmp = ptmp.tile([64, 1024], BF16, tag="tmp",
                                        name="tmp")
                        nc.vector.tensor_mul(tmp, ot[h2][0:64, :], bc_sb)
                        nc.gpsimd.dma_start(otp_sb[64:128, hp, qsl], tmp)

            def new_ot():
                return [
                    ppv.tile([DH + 1, 1024], F32, tag="ot", name=f"ot{h2}")
                    for h2 in range(2)
                ]

            # ---------------- Phase A + B(qh0, hp0) interleaved ----------
            proj_qk8(xq8, wq8_sb, qt8_sb, 0)
            proj_qk8(xq8, wq8_sb, qt8_sb, 1)
            ot00 = new_ot()
            for qc in range(QC):
                proj_qk8(xk8, wk8_sb, kt8_sb, qc)
                load_mask(4 * qc, 0)
                if qc == 0:
                    nc.gpsimd.dma_start(
                        wv_sb, wv.rearrange("(cc p) n -> p cc n", p=128))
                proj_v(qc)
                load_mask(4 * qc + 2, 0)
                if qc == 3:
                    # remaining Q projections; PE covered by exp backlog
                    proj_qk8(xq8, wq8_sb, qt8_sb, 2)
                    proj_qk8(xq8, wq8_sb, qt8_sb, 3)
                # last two k-chunks are emitted inside the sweep boundary
                b_chunk(0, 0, range(4 * qc, min(4 * qc + 4, KC - 2)), ot00)

            nc.gpsimd.dma_start(wp_sb, wp[:, :, :])

            # ---------------- Phase C building block ----------------
            def c_block(tq, tag, act_evac=False):
                trange = slice(tq * 128, (tq + 1) * 128)
                pool = pps if tag == "s" else ppv
                y_ps = pool.tile([128, 1024], F32, tag=tag, name="y_ps")
                for pair in range(2):
                    for nk in range(2):
                        ns = slice(nk * 512, (nk + 1) * 512)
                        _mm(nc, y_ps[:, ns], otp_sb[:, pair, trange],
                            wp_sb[:, pair, ns], pair == 0, pair == 1)
                y_t = py.tile([128, C], BF16, tag="y", name="y_t")
                if act_evac:
                    nc.scalar.copy(y_t, y_ps)
                else:
                    nc.vector.tensor_copy(y_t, y_ps)
                nc.gpsimd.dma_start(y[trange, :], y_t)

            # ---------------- Phase B: remaining sweeps ----------------
            # Sweep boundary: interleave the old sweep's last 2 k-chunks
            # with the new sweep's first 2 (exp never stops), drain the old
            # PVs, reciprocals, 2 more new k-chunks of S/exp/mask to cover
            # the norm chain, broadcast+evacuate, then the pipelined steady
            # state which gradually drains the new prologue PVs.
            pending_norm = (0, 0, ot00)
            for qh, hp in ((0, 1), (1, 0), (1, 1)):
                oqh, ohp, oot = pending_norm
                ot = new_ot()
                new_pend = []
                for i, kco in enumerate((KC - 2, KC - 1)):
                    for h2 in range(2):
                        pend_pv.append(
                            (ohp, kco, h2, oot, b_smask(oqh, ohp, kco, h2)))
                    b_drain(PV_LAG)
                    for h2 in range(2):
                        new_pend.append(
                            (hp, i, h2, ot, b_smask(qh, hp, i, h2)))
                    if qh == 0 and hp == 1:
                        load_mask(2 * i, 1)
                b_drain(0)
                rcs = b_recips(oot)
                for kc in range(2, 4):
                    for h2 in range(2):
                        new_pend.append(
                            (hp, kc, h2, ot, b_smask(qh, hp, kc, h2)))
                    if qh == 0 and hp == 1:
                        load_mask(2 * kc, 1)
                b_norm(oqh, ohp, oot, rcs)
                pend_pv.extend(new_pend)
                if qh == 0 and hp == 1:
                    for kc2 in range(8, KC, 2):
                        load_mask(kc2, 1)
                last = (qh, hp) == (1, 1)
                b_chunk(qh, hp, range(4, KC if last else KC - 2), ot)
                pending_norm = (qh, hp, ot)
            def b_norm_last(qh, hp, ot, rcs):
                """final-sweep norm: h2=1 first and split in halves so its
                SBUF->SBUF partition-shift DMA lands before phase C needs
                the first qh1 token blocks."""
                qsl0 = qh * 1024
                for h2 in (1, 0):
                    bc = pps.tile([64, 1024], F32, tag="s", name="bc")
                    for jj in range(2):
                        _mm(nc, bc[:, jj * 512:(jj + 1) * 512], ones_sb,
                            rcs[h2][:, jj * 512:(jj + 1) * 512], True, True)
                    bc_sb = ptmp.tile([64, 1024], F32R, tag="bc", name="bc_sb")
                    nc.vector.tensor_copy(bc_sb, bc)
                    for jj in range(2):
                        js = slice(jj * 512, (jj + 1) * 512)
                        qs = slice(qsl0 + jj * 512, qsl0 + (jj + 1) * 512)
                        if h2 == 0:
                            nc.vector.tensor_mul(otp_sb[0:64, hp, qs],
                                                 ot[h2][0:64, js], bc_sb[:, js])
                        else:
                            tmp = ptmp.tile([64, 1024], BF16, tag="tmp",
                                            name="tmp")
                            nc.vector.tensor_mul(tmp[:, js], ot[h2][0:64, js],
                                                 bc_sb[:, js])
                            nc.gpsimd.dma_start(otp_sb[64:128, hp, qs],
                                                tmp[:, js])

            b_flush()
            rcs_last = b_recips(pending_norm[2])
            # the qh0 half of the output projection depends only on sweeps
            # (0,*), finished long ago -- run it while the last norm chain
            # drains on DVE (evacuations on the now-idle ACT engine), and
            # before the norm's bc matmuls can park in PE's in-order queue
            for i, tq in enumerate(range(8)):
                c_block(tq, "s", act_evac=True)
            b_norm_last(*pending_norm, rcs_last)
            for i, tq in enumerate(range(8, T // 128)):
                c_block(tq, "s" if i % 2 == 0 else "ot", act_evac=True)

    if split_waits:
        _split_excess_waits(nc)
    return nc


_program_cache = None


def _get_program():
    global _program_cache
    if _program_cache is None:
        _program_cache = build_program()
    return _program_cache


def _dr_pack_x(xt):
    """x^T [C, T] f32 -> [128, CC//2, 2, T] fp8 with c = ccp*256+par*128+p."""
    return np.ascontiguousarray(
        xt.reshape(CC // 2, 2, 128, T).transpose(2, 0, 1, 3)
    ).astype(ml_dtypes.float8_e4m3)


def _dr_pack_w(w):
    """W slice [C, 256] f32 (pre-scaled) -> [128, CC//2, 2, 2, 128] fp8.

    Output [p, ccp, par, j, m] = W[ccp*256 + par*128 + p,
                                   (m//32)*64 + 32*j + (m%32)].
    """
    # permute columns: perm[j, m] = (m//32)*64 + 32*j + m%32
    m_idx = np.arange(128)
    perm = np.stack([(m_idx // 32) * 64 + 32 * j + (m_idx % 32)
                     for j in range(2)])            # [2, 128]
    wperm = w[:, perm]                              # [C, 2, 128]
    return np.ascontiguousarray(
        wperm.reshape(CC // 2, 2, 128, 2, 128).transpose(2, 0, 1, 3, 4)
    ).astype(ml_dtypes.float8_e4m3)


def kernel(query, key, value, mask, Wq, Wk, Wv, Wp, bp):
    query = np.asarray(query, np.float32)
    key = np.asarray(key, np.float32)
    value = np.asarray(value, np.float32)
    mask = np.asarray(mask)
    Wq = np.asarray(Wq, np.float32)
    Wk = np.asarray(Wk, np.float32)
    Wv = np.asarray(Wv, np.float32)
    Wp = np.asarray(Wp, np.float32)
    bp = np.asarray(bp, np.float32)

    in_maps = []
    for c in range(8):
        b, g = c // GROUPS, c % GROUPS
        cols = slice(g * COLS, (g + 1) * COLS)
        wp_pair = np.ascontiguousarray(
            Wp[cols, :].reshape(2, 128, C).transpose(1, 0, 2)
        ).astype(ml_dtypes.bfloat16)
        in_maps.append({
            "xq8": _dr_pack_x(query[b].T),
            "xk8": _dr_pack_x(key[b].T),
            "xvT": np.ascontiguousarray(value[b].T).astype(ml_dtypes.bfloat16),
            "maskT": np.ascontiguousarray(mask[b].T).astype(ml_dtypes.bfloat16),
            "wq8": _dr_pack_w(Wq[:, cols] * 8.0),
            "wk8": _dr_pack_w(Wk[:, cols] * 8.0),
            "wv": np.ascontiguousarray(Wv[:, cols]).astype(ml_dtypes.bfloat16),
            "wp": wp_pair,
        })

    nc = _get_program()
    res = run_bass_kernel_spmd(nc, in_maps, list(range(8)))

    out = np.empty((B, T, C), np.float32)
    for b in range(B):
        acc = res.results[b * GROUPS]["y"].astype(np.float32)
        for g in range(1, GROUPS):
            acc = acc + res.results[b * GROUPS + g]["y"].astype(np.float32)
        out[b] = acc + bp
    return out
